# revision 30
# baseline (speedup 1.0000x reference)
"""Distributed Trainium2 kernel for a dense transformer block.

Reference computation (per batch):
  x = x + o_proj(attn(rope(qkv(rmsnorm(x))), causal)) ; x = x + w2(silu(wg(rmsnorm(x))) * w1(rmsnorm(x)))

Sharding: DP=2 on batch x TP=4 on heads / MLP rows (Megatron).
Cores 0-3 handle batch 0, cores 4-7 batch 1. Within a group, rank r owns
heads 4r..4r+3 and MLP rows 1024r..1024(r+1). Two bf16 AllReduces per
group, chunked and software-pipelined against compute.

v2 schedule: per-512-token-chunk pipeline
  A(c) norm+transpose -> B(c) qkv+rope -> V(c) -> T(c) attention (skewed
  score/exp/pv) -> O(c) oproj -> AR1(c); resid(c-1)+MLP(c-1)+AR2(c-1)
  interleaved between attention chunks. rsqrt via exp(-0.5*ln(m)) keeps
  the ACT engine on one table through the attention phase; attention
  row-sum reciprocals batched across partitions.
"""

import sys

sys.path.insert(0, "/opt/trn_rl_repo")

import numpy as np
import ml_dtypes

import concourse.bass as bass
import concourse.bacc as bacc
import concourse.mybir as mybir
import concourse.tile as tile
from concourse.bass_utils import run_bass_kernel_spmd

BF = ml_dtypes.bfloat16
F32 = mybir.dt.float32
BF16 = mybir.dt.bfloat16

D = 1024
NH = 16
DH = 64
MULT = 4
EPS = 1e-5
ROPE_BASE = 10000.0
B = 2
TP = 4  # tensor-parallel ranks per group
HPC = NH // TP  # heads per core = 4
QKF = 2 * HPC * DH  # q+k shard features = 512
VF = HPC * DH  # v shard features = 256
MID = MULT * D // TP  # mlp rows per core = 1024
AF = mybir.ActivationFunctionType
ALU = mybir.AluOpType


def build_nc(T, use_silu=True):
    """Build the SPMD graph for one core (token count T per batch)."""
    DC = D // 128  # d chunks = 8
    TT = T // 128  # token tiles
    QT = min(512, T)  # q-tile width == chunk width
    NQ = T // QT  # number of chunks
    CPQ = QT // 128  # 128-token tiles per chunk
    MIDC = MID // 128  # mlp row chunks = 8
    NT = D // 512

    nc = bacc.Bacc("TRN2", target_bir_lowering=False, debug=False, num_devices=8)

    x_e = nc.dram_tensor("x", [T, D], F32, kind="ExternalInput")
    qkw_e = nc.dram_tensor("qkw_t", [D, QKF], BF16, kind="ExternalInput")
    vw_e = nc.dram_tensor("vw_m", [D, VF], BF16, kind="ExternalInput")
    ow_e = nc.dram_tensor("ow_m", [VF, D], BF16, kind="ExternalInput")
    w1w_e = nc.dram_tensor("w1w_t", [D, MID], BF16, kind="ExternalInput")
    wgw_e = nc.dram_tensor("wgw_t", [D, MID], BF16, kind="ExternalInput")
    w2w_e = nc.dram_tensor("w2w_m", [MID, D], BF16, kind="ExternalInput")
    cos_e = nc.dram_tensor("cosr", [128, T], BF16, kind="ExternalInput")
    sin_e = nc.dram_tensor("sinr", [128, T], BF16, kind="ExternalInput")
    cm_e = nc.dram_tensor("cmask", [CPQ * 128, QT], BF16, kind="ExternalInput")
    id_e = nc.dram_tensor("ident", [128, 128], BF16, kind="ExternalInput")
    out_e = nc.dram_tensor("out", [T, D], F32, kind="ExternalOutput")

    groups = [[0, 1, 2, 3], [4, 5, 6, 7]]

    with tile.TileContext(nc) as tc:
        with (
            tc.tile_pool(name="const", bufs=1) as cpool,
            tc.tile_pool(name="actfm", bufs=1) as fmpool,
            tc.tile_pool(name="qko", bufs=1) as qkpool,
            tc.tile_pool(name="afm", bufs=1) as apool,
            tc.tile_pool(name="vaug", bufs=1) as vpool,
            tc.tile_pool(name="xin", bufs=2) as xpool,
            tc.tile_pool(name="xnb", bufs=2) as xnpool,
            tc.tile_pool(name="work", bufs=4) as wpool,
            tc.tile_pool(name="rope", bufs=1) as rpool,
            tc.tile_pool(name="stats", bufs=8) as spool,
            tc.tile_pool(name="sums", bufs=1) as supool,
            tc.tile_pool(name="psA", bufs=3, space="PSUM") as psA,
            tc.tile_pool(name="psO", bufs=2, space="PSUM") as psO,
            tc.tile_pool(name="psS", bufs=2, space="PSUM") as psS,
            tc.tile_pool(name="dram", bufs=1, space="DRAM") as dpool,
        ):
            # ---- resident weights / tables ----
            def load_tiles(src, width, n, dt=BF16):
                ts = []
                for i in range(n):
                    t = cpool.tile(
                        [128, width], dt, tag=f"{src.name}_{i}", name=f"{src.name}_{i}"
                    )
                    nc.sync.dma_start(t[:], src[i * 128 : (i + 1) * 128, :])
                    ts.append(t)
                return ts

            # only what chunk 0's norm needs up front; the big weight loads
            # are emitted just-in-time inside the schedule so the x DMAs and
            # first norm/transposes aren't queued behind them
            w1r = w1w_e.rearrange("(c p) m -> p c m", p=128)
            wgr = wgw_e.rearrange("(c p) m -> p c m", p=128)
            ones64 = cpool.tile([128, 64], BF16, tag="ones64", name="ones64")
            nc.vector.memset(ones64[:], 1.0)
            ident = load_tiles(id_e, 128, 1)[0]
            epsc = cpool.tile([128, 1], F32, tag="epsc", name="epsc")
            nc.vector.memset(epsc[:], EPS)

            ar1_in = dpool.tile([T, D], BF16, name="ar1_in")
            ar1_out = dpool.tile([T, D], BF16, name="ar1_out")
            ar2_in = dpool.tile([T, D], BF16, name="ar2_in")
            ar2_out = dpool.tile([T, D], BF16, name="ar2_out")

            # ---- persistent activation tiles ----
            # xnf chunk-c columns are consumed by qk/v of chunk c before the
            # resid norm overwrites them as hnf — one physical set serves both.
            xnf = [
                fmpool.tile([128, T], BF16, tag=f"fm{d}", name=f"xnf{d}")
                for d in range(DC)
            ]
            hnf = xnf
            q_sb = [
                qkpool.tile([128, T], BF16, tag=f"qk{i}", name=f"q{i}")
                for i in range(2)
            ]
            k_sb = [
                qkpool.tile([128, T], BF16, tag=f"qk{i + 2}", name=f"k{i}")
                for i in range(2)
            ]
            O_sb = [
                qkpool.tile([128, T], BF16, tag=f"qk{i + 4}", name=f"O{i}")
                for i in range(2)
            ]
            On_sb = [
                qkpool.tile([128, T], BF16, tag=f"qk{i + 6}", name=f"On{i}")
                for i in range(2)
            ]
            a_fm = [
                apool.tile([128, T], BF16, tag=f"am{d}", name=f"a{d}")
                for d in range(MIDC)
            ]
            v_aug = [
                vpool.tile([128, HPC, DH + 1], BF16, tag=f"va{ti}", name=f"va{ti}")
                for ti in range(TT)
            ]

            # ---- norm helper: one 512-token chunk into feature-major ----
            # rsqrt via batched DVE Newton iteration: ACT only runs Square
            # (present in every table) so the ACT table never reloads.
            def norm_chunk(c, fm_tiles, src_dram, use_gpsimd):
                xns = []
                xts = []
                msb = spool.tile([128, CPQ], F32, tag="msb", name="msb", bufs=2)
                for j in range(CPQ):
                    ti = c * CPQ + j
                    xt = xpool.tile([128, D], F32, tag="xt", name="xt", bufs=4)
                    if use_gpsimd:
                        nc.gpsimd.dma_start(
                            xt[:], src_dram[ti * 128 : (ti + 1) * 128, :]
                        )
                    else:
                        nc.sync.dma_start(
                            xt[:], src_dram[ti * 128 : (ti + 1) * 128, :]
                        )
                    xn = xnpool.tile([128, D], BF16, tag="xn", name="xn", bufs=4)
                    # square's elementwise output is scratch — reuse xn, which
                    # the normalize multiply overwrites right after
                    nc.scalar.activation(
                        out=xn[:],
                        in_=xt[:],
                        func=AF.Square,
                        accum_out=msb[:, j : j + 1],
                    )
                    xns.append(xn)
                    xts.append(xt)
                # m = ss/D + eps; y = rsqrt(m) by Newton from y0 = 1/m
                # (m in ~[0.5, 4] here so convergence is fast and safe)
                m4 = spool.tile([128, CPQ], F32, tag="m4", name="m4", bufs=2)
                nc.vector.tensor_scalar(
                    m4[:], msb[:], 1.0 / D, EPS, ALU.mult, ALU.add
                )
                y4 = spool.tile([128, CPQ], F32, tag="y4", name="y4", bufs=2)
                nc.vector.reciprocal(y4[:], m4[:])
                u4 = spool.tile([128, CPQ], F32, tag="u4", name="u4", bufs=2)
                for _ in range(4):
                    nc.vector.tensor_mul(u4[:], y4[:], y4[:])
                    nc.vector.tensor_mul(u4[:], u4[:], m4[:])
                    nc.vector.tensor_scalar(
                        u4[:], u4[:], -0.5, 1.5, ALU.mult, ALU.add
                    )
                    nc.vector.tensor_mul(y4[:], y4[:], u4[:])
                for j in range(CPQ):
                    nc.vector.tensor_scalar_mul(
                        xns[j][:], xts[j][:], y4[:, j : j + 1]
                    )
                for di in range(DC):
                    tp = psS.tile([128, QT], BF16, tag="tp", name="tp", bufs=2)
                    for j, xn in enumerate(xns):
                        nc.tensor.transpose(
                            tp[:, j * 128 : (j + 1) * 128],
                            xn[:, di * 128 : (di + 1) * 128],
                            ident[:],
                        )
                    nc.vector.tensor_copy(
                        fm_tiles[di][:, c * QT : (c + 1) * QT], tp[:]
                    )

            # ---- stage B: qkv + rope for one chunk ----
            def qk_chunk(c):
                tsl = slice(c * QT, (c + 1) * QT)
                for m in range(4):  # q01 q23 k01 k23
                    dst = q_sb[m] if m < 2 else k_sb[m - 2]
                    ps = psA.tile([128, QT], F32, tag="ps", name="ps")
                    for dc in range(DC):
                        nc.tensor.matmul(
                            ps[:, :QT],
                            qkw[dc][:, m * 128 : (m + 1) * 128],
                            xnf[dc][:, tsl],
                            start=(dc == 0),
                            stop=(dc == DC - 1),
                        )
                    qb = rpool.tile([128, QT], BF16, tag="qb", name="qb")
                    nc.scalar.copy(qb[:], ps[:, :QT])
                    rot = rpool.tile([128, QT], BF16, tag="rot", name="rot")
                    for hb in (0, 64):
                        nc.vector.tensor_scalar_mul(
                            rot[hb : hb + 32, :], qb[hb + 32 : hb + 64, :], -1.0
                        )
                        nc.vector.tensor_copy(
                            rot[hb + 32 : hb + 64, :], qb[hb : hb + 32, :]
                        )
                    nc.vector.tensor_mul(qb[:], qb[:], cosr[:, tsl])
                    nc.vector.tensor_mul(rot[:], rot[:], sinr[:, tsl])
                    nc.vector.tensor_add(dst[:, tsl], qb[:], rot[:])

            def v_chunk(c):
                for ti in range(c * CPQ, (c + 1) * CPQ):
                    ps = psS.tile([128, VF], F32, tag="tp", name="psv")
                    for dc in range(DC):
                        nc.tensor.matmul(
                            ps[:],
                            xnf[dc][:, ti * 128 : (ti + 1) * 128],
                            vw[dc][:],
                            start=(dc == 0),
                            stop=(dc == DC - 1),
                        )
                    va = v_aug[ti]
                    nc.vector.tensor_copy(
                        va[:, :, 0:DH], ps.rearrange("p (h d) -> p h d", h=HPC)
                    )
                    nc.vector.memset(va[:, :, DH : DH + 1], 1.0)

            # ---- attention for one chunk: skewed score/exp/pv pipeline ----
            # `pre_units`/`units` are closures of independent PE work (resid
            # norm, MLP matmul units of the previous chunk) interleaved into
            # the attention steps: they keep the PE fed while ACT computes
            # the exps, holding the PE clock at full p-state.
            SKEW = 2

            def attn_chunk(qt, pre_units=(), units=None):
                if units is None:
                    units = []
                tsl = slice(qt * QT, (qt + 1) * QT)
                ncks = CPQ * (qt + 1)
                # row-sums of all 4 heads stacked at partitions 0/32/64/96
                sums = supool.tile([128, QT], F32, tag="sums", name="sums")
                for hp in range(2):
                    if hp == 1:
                        for u in pre_units:
                            u()
                    opsP = [
                        psO.tile([DH + 1, QT], F32, tag="pso", name=f"ops{i}")
                        for i in range(2)
                    ]

                    def emit_score(ck):
                        pts = []
                        for i in range(2):
                            hb = i * 64
                            sp = psA.tile([128, QT], F32, tag="ps", name="sp")
                            nc.tensor.matmul(
                                sp[:, :QT],
                                k_sb[hp][hb : hb + DH, ck * 128 : (ck + 1) * 128],
                                q_sb[hp][hb : hb + DH, tsl],
                                start=True,
                                stop=True,
                            )
                            pt = wpool.tile(
                                [128, QT], BF16, tag="pt", name="pt", bufs=4
                            )
                            j = ck - CPQ * qt
                            if j > 0:
                                lo = j * 128
                                nc.vector.memset(pt[:, :lo], 0.0)
                                nc.scalar.activation(
                                    out=pt[:, lo:],
                                    in_=sp[:, lo:QT],
                                    func=AF.Exp,
                                    scale=0.125,
                                )
                                nc.vector.tensor_mul(
                                    pt[:, lo:], pt[:, lo:], cmask[:, : QT - lo]
                                )
                            else:
                                nc.scalar.activation(
                                    out=pt[:], in_=sp[:, :QT], func=AF.Exp, scale=0.125
                                )
                                if j == 0:
                                    nc.vector.tensor_mul(pt[:], pt[:], cmask[:])
                            pts.append(pt)
                        return pts

                    def emit_pv(ck, pts):
                        for i in range(2):
                            nc.tensor.matmul(
                                opsP[i][:],
                                v_aug[ck][:, 2 * hp + i, :],
                                pts[i][:],
                                start=(ck == 0),
                                stop=(ck == ncks - 1),
                            )

                    buf = {}
                    for ck in range(ncks):
                        buf[ck] = emit_score(ck)
                        if ck >= SKEW:
                            emit_pv(ck - SKEW, buf.pop(ck - SKEW))
                        if hp == 1 and units:
                            units.pop(0)()
                    for ck in range(max(0, ncks - SKEW), ncks):
                        emit_pv(ck, buf.pop(ck))

                    for i in range(2):
                        h = 2 * hp + i
                        nc.scalar.copy(
                            sums[32 * h : 32 * h + 1, :], opsP[i][DH : DH + 1, :]
                        )
                        nc.scalar.copy(
                            O_sb[hp][i * 64 : i * 64 + DH, tsl], opsP[i][0:DH, :]
                        )
                # one batched in-place reciprocal for all 4 heads
                rin = sums
                nc.vector.reciprocal(rin[:], sums[:])
                # cast to bf16 with a partition shift so every head's row sits
                # at a legal matmul base partition (0 or 32)
                rinb = [
                    wpool.tile([128, QT], BF16, tag="pt", name=f"rinb{hp}", bufs=4)
                    for hp in range(2)
                ]
                nc.vector.tensor_copy(rinb[0][0:64, :], rin[0:64, :])
                nc.vector.tensor_copy(rinb[1][0:64, :], rin[64:128, :])
                for hp in range(2):
                    bb = psS.tile([128, QT], F32, tag="bb", name="bb", bufs=1)
                    for i in range(2):
                        nc.tensor.matmul(
                            bb[i * 64 : (i + 1) * 64, :QT],
                            ones64[32 * i : 32 * i + 1, :],
                            rinb[hp][32 * i : 32 * i + 1, :],
                            start=True,
                            stop=True,
                        )
                    nc.vector.tensor_mul(
                        On_sb[hp][:, tsl], O_sb[hp][:, tsl], bb[:, :QT]
                    )

            # ---- o-proj + AR1 input for one chunk ----
            def oproj_chunk(c):
                for ti in range(c * CPQ, (c + 1) * CPQ):
                    ob = wpool.tile([128, D], BF16, tag="ob", name="ob", bufs=2)
                    xo = xpool.tile([128, D], F32, tag="xo", name="xo")
                    nc.sync.dma_start(xo[:], x_e[ti * 128 : (ti + 1) * 128, :])
                    for nt in range(NT):
                        ps = psA.tile([128, QT], F32, tag="ps", name="ps")
                        for cc in range(VF // 128):
                            nc.tensor.matmul(
                                ps[:, :512],
                                On_sb[cc][:, ti * 128 : (ti + 1) * 128],
                                ow[cc][:, nt * 512 : (nt + 1) * 512],
                                start=(cc == 0),
                                stop=(cc == VF // 128 - 1),
                            )
                        nc.vector.scalar_tensor_tensor(
                            ob[:, nt * 512 : (nt + 1) * 512],
                            xo[:, nt * 512 : (nt + 1) * 512],
                            1.0 / TP,
                            ps[:, :512],
                            ALU.mult,
                            ALU.add,
                        )
                    nc.sync.dma_start(ar1_in[ti * 128 : (ti + 1) * 128, :], ob[:])

            def ar1_fire(lo, hi):
                nc.gpsimd.collective_compute(
                    "AllReduce",
                    ALU.add,
                    ins=[ar1_in[lo:hi, :].opt()],
                    outs=[ar1_out[lo:hi, :].opt()],
                    replica_groups=groups,
                )

            # ---- MLP for one chunk, as independent interleavable units ----
            def mlp_mc(c, mc):
                tsl = slice(c * QT, (c + 1) * QT)
                msl = slice(mc * 128, (mc + 1) * 128)
                wg_mc = wpool.tile(
                    [128, DC, 128], BF16, tag="wgs", name="wg_mc", bufs=2
                )
                nc.sync.dma_start(wg_mc[:], wgr[:, :, msl])
                w1_mc = wpool.tile(
                    [128, DC, 128], BF16, tag="w1s", name="w1_mc", bufs=2
                )
                nc.sync.dma_start(w1_mc[:], w1r[:, :, msl])
                psg = psA.tile([128, QT], F32, tag="ps", name="psg")
                for dc in range(DC):
                    nc.tensor.matmul(
                        psg[:, :QT],
                        wg_mc[:, dc, :],
                        hnf[dc][:, tsl],
                        start=(dc == 0),
                        stop=(dc == DC - 1),
                    )
                # silu via tanh (same ACT table as attention's exp, so the
                # interleave never reloads tables):
                #   silu(x) = x*0.5*(1+tanh(x/2)) -> g' = (tanh+1)*x; a = 0.5*g'*u
                th = rpool.tile([128, QT], BF16, tag="rot", name="th", bufs=1)
                nc.scalar.activation(
                    out=th[:], in_=psg[:, :QT], func=AF.Tanh, scale=0.5
                )
                g_sb = rpool.tile([128, QT], BF16, tag="qb", name="g2", bufs=1)
                nc.vector.scalar_tensor_tensor(
                    g_sb[:], th[:], 1.0, psg[:, :QT], ALU.add, ALU.mult
                )
                psu = psA.tile([128, QT], F32, tag="ps", name="psu")
                for dc in range(DC):
                    nc.tensor.matmul(
                        psu[:, :QT],
                        w1_mc[:, dc, :],
                        hnf[dc][:, tsl],
                        start=(dc == 0),
                        stop=(dc == DC - 1),
                    )
                nc.vector.scalar_tensor_tensor(
                    a_fm[mc][:, tsl], g_sb[:], 0.5, psu[:, :QT], ALU.mult, ALU.mult
                )

            def w2_ti(ti):
                ob = wpool.tile([128, D], BF16, tag="ob", name="ob", bufs=2)
                h1t = xpool.tile([128, D], F32, tag="h1t", name="h1t")
                nc.gpsimd.dma_start(h1t[:], ar1_out[ti * 128 : (ti + 1) * 128, :])
                for nt in range(NT):
                    ps = psA.tile([128, QT], F32, tag="ps", name="ps")
                    for mc in range(MIDC):
                        nc.tensor.matmul(
                            ps[:, :512],
                            a_fm[mc][:, ti * 128 : (ti + 1) * 128],
                            w2w[mc][:, nt * 512 : (nt + 1) * 512],
                            start=(mc == 0),
                            stop=(mc == MIDC - 1),
                        )
                    nc.vector.scalar_tensor_tensor(
                        ob[:, nt * 512 : (nt + 1) * 512],
                        h1t[:, nt * 512 : (nt + 1) * 512],
                        1.0 / TP,
                        ps[:, :512],
                        ALU.mult,
                        ALU.add,
                    )
                nc.sync.dma_start(ar2_in[ti * 128 : (ti + 1) * 128, :], ob[:])

            def mlp_units(c):
                us = [
                    (lambda mc=mc: mlp_mc(c, mc)) for mc in range(MIDC)
                ]
                us += [
                    (lambda ti=ti: w2_ti(ti))
                    for ti in range(c * CPQ, (c + 1) * CPQ)
                ]
                return us

            def mlp_chunk(c):
                for u in mlp_units(c):
                    u()

            def w2_chunk(c):
                pass

            def ar2_fire(lo, hi):
                nc.gpsimd.collective_compute(
                    "AllReduce",
                    ALU.add,
                    ins=[ar2_in[lo:hi, :].opt()],
                    outs=[ar2_out[lo:hi, :].opt()],
                    replica_groups=groups,
                )

            def final_piece(lo, hi):
                nc.gpsimd.dma_start(out_e[lo:hi, :], ar2_out[lo:hi, :])

            # ---- schedule ----
            if NQ == 1:
                norm_chunk(0, xnf, x_e, False)
                qkw = load_tiles(qkw_e, QKF, DC)
                cosr = load_tiles(cos_e, T, 1)[0]
                sinr = load_tiles(sin_e, T, 1)[0]
                vw = load_tiles(vw_e, VF, DC)
                cmask = load_tiles(cm_e, QT, 1)[0]
                ow = load_tiles(ow_e, D, VF // 128)
                w2w = load_tiles(w2w_e, D, MIDC)
                qk_chunk(0)
                v_chunk(0)
                attn_chunk(0)
                oproj_chunk(0)
                ar1_fire(0, T)
                norm_chunk(0, hnf, ar1_out, True)
                mlp_chunk(0)
                w2_chunk(0)
                ar2_fire(0, T)
                final_piece(0, T)
            else:
                for c in range(NQ):
                    norm_chunk(c, xnf, x_e, False)
                    if c == 0:
                        qkw = load_tiles(qkw_e, QKF, DC)
                        cosr = load_tiles(cos_e, T, 1)[0]
                        sinr = load_tiles(sin_e, T, 1)[0]
                    qk_chunk(c)
                    if c == 0:
                        vw = load_tiles(vw_e, VF, DC)
                    v_chunk(c)
                    if c == 0:
                        cmask = load_tiles(cm_e, QT, 1)[0]
                    # previous chunk's resid norm + MLP ride inside the
                    # attention steps to keep the PE from draining while
                    # ACT computes the exps
                    if c >= 1:
                        k = c - 1
                        pre = [lambda k=k: norm_chunk(k, hnf, ar1_out, True)]
                        units = mlp_units(k)
                    else:
                        pre, units = [], []
                    attn_chunk(c, pre, units)
                    if c == 0:
                        ow = load_tiles(ow_e, D, VF // 128)
                    oproj_chunk(c)
                    ar1_fire(c * QT, (c + 1) * QT)
                    if c == 0:
                        w2w = load_tiles(w2w_e, D, MIDC)
                    if c >= 1:
                        k = c - 1
                        for u in units:  # spill units that didn't fit
                            u()
                        ar2_fire(k * QT, (k + 1) * QT)
                        if k >= 1:
                            final_piece((k - 1) * QT, k * QT)
                # tail: last chunk of MLP
                k = NQ - 1
                norm_chunk(k, hnf, ar1_out, True)
                mlp_chunk(k)
                ar2_fire(k * QT, (k + 1) * QT)
                final_piece((k - 1) * QT, k * QT)
                final_piece(k * QT, (k + 1) * QT)

    nc.compile()
    return nc


def make_in_maps(x, n1_w, n2_w, qkv_w, o_w, w1_w, wg_w, w2_w, T):
    QT = min(512, T)
    CPQ = QT // 128
    half = DH // 2
    freqs = np.arange(half, dtype=np.float64) / half
    theta = 1.0 / ROPE_BASE**freqs
    ang = np.arange(T, dtype=np.float64)[:, None] * theta[None, :]  # [T, 32]
    p = np.arange(128) % half
    cosr = np.cos(ang)[:, p].T.astype(BF)  # [128, T]
    sinr = np.sin(ang)[:, p].T.astype(BF)
    cm = np.zeros((CPQ * 128, QT), dtype=BF)
    for j in range(CPQ):
        tk = np.arange(128)[:, None]
        tq = np.arange(QT)[None, :]
        cm[j * 128 : (j + 1) * 128] = (tq >= j * 128 + tk).astype(BF)

    in_maps = []
    for c in range(8):
        b, r = c // 4, c % 4
        qs = slice(r * VF, (r + 1) * VF)
        qr = qkv_w[0 * D :][qs] * n1_w[None, :]
        kr = qkv_w[1 * D :][qs] * n1_w[None, :]
        vr = qkv_w[2 * D :][qs] * n1_w[None, :]
        ms = slice(r * MID, (r + 1) * MID)
        in_maps.append(
            {
                "x": np.ascontiguousarray(x[b, :T], np.float32),
                "qkw_t": np.ascontiguousarray(
                    np.concatenate([qr, kr], 0).T.astype(BF)
                ),
                "vw_m": np.ascontiguousarray(vr.T.astype(BF)),
                "ow_m": np.ascontiguousarray(o_w[:, qs].T.astype(BF)),
                "w1w_t": np.ascontiguousarray(
                    (w1_w[ms] * n2_w[None, :]).T.astype(BF)
                ),
                "wgw_t": np.ascontiguousarray(
                    (wg_w[ms] * n2_w[None, :]).T.astype(BF)
                ),
                "w2w_m": np.ascontiguousarray(w2_w[:, ms].T.astype(BF)),
                "cosr": cosr,
                "sinr": sinr,
                "cmask": cm,
                "ident": np.eye(128, dtype=BF),
            }
        )
    return in_maps


_CACHE = {}


def _get_nc(T):
    if T not in _CACHE:
        _CACHE[T] = build_nc(T, use_silu=True)
    return _CACHE[T]


def run(inputs, T=2048, trace=False):
    nc = _get_nc(T)
    in_maps = make_in_maps(T=T, **inputs)
    res = run_bass_kernel_spmd(nc, in_maps, core_ids=list(range(8)), trace=trace)
    out = np.stack([res.results[0]["out"], res.results[4]["out"]])
    return out, res


def kernel(**inputs):
    out, _ = run(inputs, T=2048)
    return out


# revision 35
# speedup vs baseline: 1.0637x; 1.0637x over previous
"""Distributed Trainium2 kernel for a dense transformer block.

Reference computation (per batch):
  x = x + o_proj(attn(rope(qkv(rmsnorm(x))), causal)) ; x = x + w2(silu(wg(rmsnorm(x))) * w1(rmsnorm(x)))

Sharding: DP=2 on batch x TP=4 on heads / MLP rows (Megatron).
Cores 0-3 handle batch 0, cores 4-7 batch 1. Within a group, rank r owns
heads 4r..4r+3 and MLP rows 1024r..1024(r+1). Two bf16 AllReduces per
group, chunked and software-pipelined against compute.

v2 schedule: per-512-token-chunk pipeline
  A(c) norm+transpose -> B(c) qkv+rope -> V(c) -> T(c) attention (skewed
  score/exp/pv) -> O(c) oproj -> AR1(c); resid(c-1)+MLP(c-1)+AR2(c-1)
  interleaved between attention chunks. rsqrt via exp(-0.5*ln(m)) keeps
  the ACT engine on one table through the attention phase; attention
  row-sum reciprocals batched across partitions.
"""

import sys

sys.path.insert(0, "/opt/trn_rl_repo")

import numpy as np
import ml_dtypes

import concourse.bass as bass
import concourse.bacc as bacc
import concourse.mybir as mybir
import concourse.tile as tile
from concourse.bass_utils import run_bass_kernel_spmd

BF = ml_dtypes.bfloat16
F32 = mybir.dt.float32
BF16 = mybir.dt.bfloat16

D = 1024
NH = 16
DH = 64
MULT = 4
EPS = 1e-5
ROPE_BASE = 10000.0
B = 2
TP = 4  # tensor-parallel ranks per group
HPC = NH // TP  # heads per core = 4
QKF = 2 * HPC * DH  # q+k shard features = 512
VF = HPC * DH  # v shard features = 256
MID = MULT * D // TP  # mlp rows per core = 1024
AF = mybir.ActivationFunctionType
ALU = mybir.AluOpType


def build_nc(T, use_silu=True):
    """Build the SPMD graph for one core (token count T per batch)."""
    DC = D // 128  # d chunks = 8
    TT = T // 128  # token tiles
    QT = min(512, T)  # q-tile width == chunk width
    NQ = T // QT  # number of chunks
    CPQ = QT // 128  # 128-token tiles per chunk
    MIDC = MID // 128  # mlp row chunks = 8
    NT = D // 512

    nc = bacc.Bacc("TRN2", target_bir_lowering=False, debug=False, num_devices=8)

    x_e = nc.dram_tensor("x", [T, D], F32, kind="ExternalInput")
    qkw_e = nc.dram_tensor("qkw_t", [D, QKF], BF16, kind="ExternalInput")
    vw_e = nc.dram_tensor("vw_m", [D, VF], BF16, kind="ExternalInput")
    ow_e = nc.dram_tensor("ow_m", [VF, D], BF16, kind="ExternalInput")
    w1w_e = nc.dram_tensor("w1w_t", [D, MID], BF16, kind="ExternalInput")
    wgw_e = nc.dram_tensor("wgw_t", [D, MID], BF16, kind="ExternalInput")
    w2w_e = nc.dram_tensor("w2w_m", [MID, D], BF16, kind="ExternalInput")
    cos_e = nc.dram_tensor("cosr", [128, T], BF16, kind="ExternalInput")
    sin_e = nc.dram_tensor("sinr", [128, T], BF16, kind="ExternalInput")
    cm_e = nc.dram_tensor("cmask", [CPQ * 128, QT], BF16, kind="ExternalInput")
    id_e = nc.dram_tensor("ident", [128, 128], BF16, kind="ExternalInput")
    out_e = nc.dram_tensor("out", [T, D], F32, kind="ExternalOutput")

    groups = [[0, 1, 2, 3], [4, 5, 6, 7]]

    with tile.TileContext(nc) as tc:
        with (
            tc.tile_pool(name="const", bufs=1) as cpool,
            tc.tile_pool(name="actfm", bufs=1) as fmpool,
            tc.tile_pool(name="qko", bufs=1) as qkpool,
            tc.tile_pool(name="afm", bufs=1) as apool,
            tc.tile_pool(name="vaug", bufs=1) as vpool,
            tc.tile_pool(name="xin", bufs=2) as xpool,
            tc.tile_pool(name="xnb", bufs=2) as xnpool,
            tc.tile_pool(name="work", bufs=4) as wpool,
            tc.tile_pool(name="rope", bufs=1) as rpool,
            tc.tile_pool(name="stats", bufs=8) as spool,
            tc.tile_pool(name="sums", bufs=1) as supool,
            tc.tile_pool(name="psA", bufs=3, space="PSUM") as psA,
            tc.tile_pool(name="psO", bufs=2, space="PSUM") as psO,
            tc.tile_pool(name="psS", bufs=2, space="PSUM") as psS,
            tc.tile_pool(name="dram", bufs=1, space="DRAM") as dpool,
        ):
            # ---- resident weights / tables ----
            def load_tiles(src, width, n, dt=BF16):
                ts = []
                for i in range(n):
                    t = cpool.tile(
                        [128, width], dt, tag=f"{src.name}_{i}", name=f"{src.name}_{i}"
                    )
                    nc.sync.dma_start(t[:], src[i * 128 : (i + 1) * 128, :])
                    ts.append(t)
                return ts

            # only what chunk 0's norm needs up front; the big weight loads
            # are emitted just-in-time inside the schedule so the x DMAs and
            # first norm/transposes aren't queued behind them
            w1r = w1w_e.rearrange("(c p) m -> p c m", p=128)
            wgr = wgw_e.rearrange("(c p) m -> p c m", p=128)
            ones64 = cpool.tile([128, 64], BF16, tag="ones64", name="ones64")
            nc.vector.memset(ones64[:], 1.0)
            ident = load_tiles(id_e, 128, 1)[0]
            epsc = cpool.tile([128, 1], F32, tag="epsc", name="epsc")
            nc.vector.memset(epsc[:], EPS)

            ar1_in = dpool.tile([T, D], BF16, name="ar1_in")
            ar1_out = dpool.tile([T, D], BF16, name="ar1_out")
            ar2_in = dpool.tile([T, D], BF16, name="ar2_in")
            ar2_out = dpool.tile([T, D], BF16, name="ar2_out")

            # ---- persistent activation tiles ----
            # xnf chunk-c columns are consumed by qk/v of chunk c before the
            # resid norm overwrites them as hnf — one physical set serves both.
            xnf = [
                fmpool.tile([128, T], BF16, tag=f"fm{d}", name=f"xnf{d}")
                for d in range(DC)
            ]
            hnf = xnf
            q_sb = [
                qkpool.tile([128, T], BF16, tag=f"qk{i}", name=f"q{i}")
                for i in range(2)
            ]
            k_sb = [
                qkpool.tile([128, T], BF16, tag=f"qk{i + 2}", name=f"k{i}")
                for i in range(2)
            ]
            O_sb = [
                qkpool.tile([128, T], BF16, tag=f"qk{i + 4}", name=f"O{i}")
                for i in range(2)
            ]
            On_sb = [
                qkpool.tile([128, T], BF16, tag=f"qk{i + 6}", name=f"On{i}")
                for i in range(2)
            ]
            a_fm = [
                apool.tile([128, T], BF16, tag=f"am{d}", name=f"a{d}")
                for d in range(MIDC)
            ]
            v_aug = [
                vpool.tile([128, HPC, DH + 1], BF16, tag=f"va{ti}", name=f"va{ti}")
                for ti in range(TT)
            ]

            # ---- norm helper: one 512-token chunk into feature-major ----
            # rsqrt via batched DVE Newton iteration: ACT only runs Square
            # (present in every table) so the ACT table never reloads.
            def norm_chunk(c, fm_tiles, src_dram, use_gpsimd):
                xns = []
                xts = []
                msb = spool.tile([128, CPQ], F32, tag="msb", name="msb", bufs=2)
                for j in range(CPQ):
                    ti = c * CPQ + j
                    xt = xpool.tile([128, D], F32, tag="xt", name="xt", bufs=4)
                    if use_gpsimd:
                        nc.gpsimd.dma_start(
                            xt[:], src_dram[ti * 128 : (ti + 1) * 128, :]
                        )
                    else:
                        nc.sync.dma_start(
                            xt[:], src_dram[ti * 128 : (ti + 1) * 128, :]
                        )
                    xn = xnpool.tile([128, D], BF16, tag="xn", name="xn", bufs=4)
                    # square's elementwise output is scratch — reuse xn, which
                    # the normalize multiply overwrites right after
                    nc.scalar.activation(
                        out=xn[:],
                        in_=xt[:],
                        func=AF.Square,
                        accum_out=msb[:, j : j + 1],
                    )
                    xns.append(xn)
                    xts.append(xt)
                # m = ss/D + eps; y = rsqrt(m) by Newton from y0 = 1/m
                # (m in ~[0.5, 4] here so convergence is fast and safe)
                m4 = spool.tile([128, CPQ], F32, tag="m4", name="m4", bufs=2)
                nc.vector.tensor_scalar(
                    m4[:], msb[:], 1.0 / D, EPS, ALU.mult, ALU.add
                )
                y4 = spool.tile([128, CPQ], F32, tag="y4", name="y4", bufs=2)
                nc.vector.reciprocal(y4[:], m4[:])
                u4 = spool.tile([128, CPQ], F32, tag="u4", name="u4", bufs=2)
                for _ in range(4):
                    nc.vector.tensor_mul(u4[:], y4[:], y4[:])
                    nc.vector.tensor_mul(u4[:], u4[:], m4[:])
                    nc.vector.tensor_scalar(
                        u4[:], u4[:], -0.5, 1.5, ALU.mult, ALU.add
                    )
                    nc.vector.tensor_mul(y4[:], y4[:], u4[:])
                for j in range(CPQ):
                    nc.vector.tensor_scalar_mul(
                        xns[j][:], xts[j][:], y4[:, j : j + 1]
                    )
                for di in range(DC):
                    tp = psS.tile([128, QT], BF16, tag="tp", name="tp", bufs=2)
                    for j, xn in enumerate(xns):
                        nc.tensor.transpose(
                            tp[:, j * 128 : (j + 1) * 128],
                            xn[:, di * 128 : (di + 1) * 128],
                            ident[:],
                        )
                    nc.vector.tensor_copy(
                        fm_tiles[di][:, c * QT : (c + 1) * QT], tp[:]
                    )

            # ---- stage B: qkv + rope for one chunk ----
            def qk_chunk(c):
                tsl = slice(c * QT, (c + 1) * QT)
                for m in range(4):  # q01 q23 k01 k23
                    dst = q_sb[m] if m < 2 else k_sb[m - 2]
                    ps = psA.tile([128, QT], F32, tag="ps", name="ps")
                    for dc in range(DC):
                        nc.tensor.matmul(
                            ps[:, :QT],
                            qkw[dc][:, m * 128 : (m + 1) * 128],
                            xnf[dc][:, tsl],
                            start=(dc == 0),
                            stop=(dc == DC - 1),
                        )
                    qb = rpool.tile([128, QT], BF16, tag="qb", name="qb")
                    nc.scalar.copy(qb[:], ps[:, :QT])
                    rot = rpool.tile([128, QT], BF16, tag="rot", name="rot")
                    for hb in (0, 64):
                        nc.vector.tensor_scalar_mul(
                            rot[hb : hb + 32, :], qb[hb + 32 : hb + 64, :], -1.0
                        )
                        nc.vector.tensor_copy(
                            rot[hb + 32 : hb + 64, :], qb[hb : hb + 32, :]
                        )
                    nc.vector.tensor_mul(qb[:], qb[:], cosr[:, tsl])
                    nc.vector.tensor_mul(rot[:], rot[:], sinr[:, tsl])
                    nc.vector.tensor_add(dst[:, tsl], qb[:], rot[:])

            def v_chunk(c):
                for ti in range(c * CPQ, (c + 1) * CPQ):
                    ps = psS.tile([128, VF], F32, tag="tp", name="psv")
                    for dc in range(DC):
                        nc.tensor.matmul(
                            ps[:],
                            xnf[dc][:, ti * 128 : (ti + 1) * 128],
                            vw[dc][:],
                            start=(dc == 0),
                            stop=(dc == DC - 1),
                        )
                    va = v_aug[ti]
                    nc.vector.tensor_copy(
                        va[:, :, 0:DH], ps.rearrange("p (h d) -> p h d", h=HPC)
                    )
                    nc.vector.memset(va[:, :, DH : DH + 1], 1.0)

            # ---- attention for one chunk: skewed score/exp/pv pipeline ----
            # `pre_units`/`units` are closures of independent PE work (resid
            # norm, MLP matmul units of the previous chunk) interleaved into
            # the attention steps: they keep the PE fed while ACT computes
            # the exps, holding the PE clock at full p-state.
            SKEW = 2

            def attn_chunk(qt, pre_units=(), units=None, max_units=99):
                if units is None:
                    units = []
                popped = 0
                tsl = slice(qt * QT, (qt + 1) * QT)
                ncks = CPQ * (qt + 1)
                # row-sums of all 4 heads stacked at partitions 0/32/64/96
                sums = supool.tile([128, QT], F32, tag="sums", name="sums")
                for hp in range(2):
                    if hp == 1:
                        for u in pre_units:
                            u()
                    opsP = [
                        psO.tile([DH + 1, QT], F32, tag="pso", name=f"ops{i}")
                        for i in range(2)
                    ]

                    def emit_score(ck):
                        pts = []
                        for i in range(2):
                            hb = i * 64
                            sp = psA.tile([128, QT], F32, tag="ps", name="sp")
                            nc.tensor.matmul(
                                sp[:, :QT],
                                k_sb[hp][hb : hb + DH, ck * 128 : (ck + 1) * 128],
                                q_sb[hp][hb : hb + DH, tsl],
                                start=True,
                                stop=True,
                            )
                            pt = wpool.tile(
                                [128, QT], BF16, tag="pt", name="pt", bufs=4
                            )
                            j = ck - CPQ * qt
                            if j > 0:
                                lo = j * 128
                                nc.vector.memset(pt[:, :lo], 0.0)
                                nc.scalar.activation(
                                    out=pt[:, lo:],
                                    in_=sp[:, lo:QT],
                                    func=AF.Exp,
                                    scale=0.125,
                                )
                                nc.vector.tensor_mul(
                                    pt[:, lo:], pt[:, lo:], cmask[:, : QT - lo]
                                )
                            else:
                                nc.scalar.activation(
                                    out=pt[:], in_=sp[:, :QT], func=AF.Exp, scale=0.125
                                )
                                if j == 0:
                                    nc.vector.tensor_mul(pt[:], pt[:], cmask[:])
                            pts.append(pt)
                        return pts

                    def emit_pv(ck, pts):
                        for i in range(2):
                            nc.tensor.matmul(
                                opsP[i][:],
                                v_aug[ck][:, 2 * hp + i, :],
                                pts[i][:],
                                start=(ck == 0),
                                stop=(ck == ncks - 1),
                            )

                    buf = {}
                    for ck in range(ncks):
                        buf[ck] = emit_score(ck)
                        if ck >= SKEW:
                            emit_pv(ck - SKEW, buf.pop(ck - SKEW))
                        if hp == 1 and units and popped < max_units:
                            units.pop(0)()
                            popped += 1
                    for ck in range(max(0, ncks - SKEW), ncks):
                        emit_pv(ck, buf.pop(ck))

                    for i in range(2):
                        h = 2 * hp + i
                        nc.scalar.copy(
                            sums[32 * h : 32 * h + 1, :], opsP[i][DH : DH + 1, :]
                        )
                        nc.scalar.copy(
                            O_sb[hp][i * 64 : i * 64 + DH, tsl], opsP[i][0:DH, :]
                        )
                # one batched in-place reciprocal for all 4 heads
                rin = sums
                nc.vector.reciprocal(rin[:], sums[:])
                # cast to bf16 with a partition shift so every head's row sits
                # at a legal matmul base partition (0 or 32)
                rinb = [
                    wpool.tile([128, QT], BF16, tag="pt", name=f"rinb{hp}", bufs=4)
                    for hp in range(2)
                ]
                nc.vector.tensor_copy(rinb[0][0:64, :], rin[0:64, :])
                nc.vector.tensor_copy(rinb[1][0:64, :], rin[64:128, :])
                for hp in range(2):
                    bb = psS.tile([128, QT], F32, tag="bb", name="bb", bufs=1)
                    for i in range(2):
                        nc.tensor.matmul(
                            bb[i * 64 : (i + 1) * 64, :QT],
                            ones64[32 * i : 32 * i + 1, :],
                            rinb[hp][32 * i : 32 * i + 1, :],
                            start=True,
                            stop=True,
                        )
                    nc.vector.tensor_mul(
                        On_sb[hp][:, tsl], O_sb[hp][:, tsl], bb[:, :QT]
                    )

            # ---- o-proj + AR1 input for one chunk ----
            def oproj_chunk(c):
                for ti in range(c * CPQ, (c + 1) * CPQ):
                    ob = wpool.tile([128, D], BF16, tag="ob", name="ob", bufs=2)
                    xo = xpool.tile([128, D], F32, tag="xo", name="xo")
                    nc.sync.dma_start(xo[:], x_e[ti * 128 : (ti + 1) * 128, :])
                    for nt in range(NT):
                        ps = psA.tile([128, QT], F32, tag="ps", name="ps")
                        for cc in range(VF // 128):
                            nc.tensor.matmul(
                                ps[:, :512],
                                On_sb[cc][:, ti * 128 : (ti + 1) * 128],
                                ow[cc][:, nt * 512 : (nt + 1) * 512],
                                start=(cc == 0),
                                stop=(cc == VF // 128 - 1),
                            )
                        nc.vector.scalar_tensor_tensor(
                            ob[:, nt * 512 : (nt + 1) * 512],
                            xo[:, nt * 512 : (nt + 1) * 512],
                            1.0 / TP,
                            ps[:, :512],
                            ALU.mult,
                            ALU.add,
                        )
                    nc.sync.dma_start(ar1_in[ti * 128 : (ti + 1) * 128, :], ob[:])

            def ar1_fire(lo, hi):
                nc.gpsimd.collective_compute(
                    "AllReduce",
                    ALU.add,
                    ins=[ar1_in[lo:hi, :].opt()],
                    outs=[ar1_out[lo:hi, :].opt()],
                    replica_groups=groups,
                )

            # ---- MLP for one chunk, as independent interleavable units ----
            def mlp_mc(c, mc):
                tsl = slice(c * QT, (c + 1) * QT)
                msl = slice(mc * 128, (mc + 1) * 128)
                wg_mc = wpool.tile(
                    [128, DC, 128], BF16, tag="wgs", name="wg_mc", bufs=2
                )
                nc.sync.dma_start(wg_mc[:], wgr[:, :, msl])
                w1_mc = wpool.tile(
                    [128, DC, 128], BF16, tag="w1s", name="w1_mc", bufs=2
                )
                nc.sync.dma_start(w1_mc[:], w1r[:, :, msl])
                psg = psA.tile([128, QT], F32, tag="ps", name="psg")
                for dc in range(DC):
                    nc.tensor.matmul(
                        psg[:, :QT],
                        wg_mc[:, dc, :],
                        hnf[dc][:, tsl],
                        start=(dc == 0),
                        stop=(dc == DC - 1),
                    )
                # silu via tanh (same ACT table as attention's exp, so the
                # interleave never reloads tables):
                #   silu(x) = x*0.5*(1+tanh(x/2)) -> g' = (tanh+1)*x; a = 0.5*g'*u
                th = rpool.tile([128, QT], BF16, tag="rot", name="th", bufs=1)
                nc.scalar.activation(
                    out=th[:], in_=psg[:, :QT], func=AF.Tanh, scale=0.5
                )
                g_sb = rpool.tile([128, QT], BF16, tag="qb", name="g2", bufs=1)
                nc.vector.scalar_tensor_tensor(
                    g_sb[:], th[:], 1.0, psg[:, :QT], ALU.add, ALU.mult
                )
                psu = psA.tile([128, QT], F32, tag="ps", name="psu")
                for dc in range(DC):
                    nc.tensor.matmul(
                        psu[:, :QT],
                        w1_mc[:, dc, :],
                        hnf[dc][:, tsl],
                        start=(dc == 0),
                        stop=(dc == DC - 1),
                    )
                nc.vector.scalar_tensor_tensor(
                    a_fm[mc][:, tsl], g_sb[:], 0.5, psu[:, :QT], ALU.mult, ALU.mult
                )

            def w2_ti(ti):
                ob = wpool.tile([128, D], BF16, tag="ob", name="ob", bufs=2)
                h1t = xpool.tile([128, D], F32, tag="h1t", name="h1t")
                nc.gpsimd.dma_start(h1t[:], ar1_out[ti * 128 : (ti + 1) * 128, :])
                for nt in range(NT):
                    ps = psA.tile([128, QT], F32, tag="ps", name="ps")
                    for mc in range(MIDC):
                        nc.tensor.matmul(
                            ps[:, :512],
                            a_fm[mc][:, ti * 128 : (ti + 1) * 128],
                            w2w[mc][:, nt * 512 : (nt + 1) * 512],
                            start=(mc == 0),
                            stop=(mc == MIDC - 1),
                        )
                    nc.vector.scalar_tensor_tensor(
                        ob[:, nt * 512 : (nt + 1) * 512],
                        h1t[:, nt * 512 : (nt + 1) * 512],
                        1.0 / TP,
                        ps[:, :512],
                        ALU.mult,
                        ALU.add,
                    )
                nc.sync.dma_start(ar2_in[ti * 128 : (ti + 1) * 128, :], ob[:])

            def mlp_units(c):
                us = [
                    (lambda mc=mc: mlp_mc(c, mc)) for mc in range(MIDC)
                ]
                us += [
                    (lambda ti=ti: w2_ti(ti))
                    for ti in range(c * CPQ, (c + 1) * CPQ)
                ]
                return us

            def mlp_chunk(c):
                for u in mlp_units(c):
                    u()

            def w2_chunk(c):
                pass

            def ar2_fire(lo, hi):
                nc.gpsimd.collective_compute(
                    "AllReduce",
                    ALU.add,
                    ins=[ar2_in[lo:hi, :].opt()],
                    outs=[ar2_out[lo:hi, :].opt()],
                    replica_groups=groups,
                )

            def final_piece(lo, hi):
                # gpsimd: the bf16->f32 cast DMA is gpsimd-only. Emission
                # points are one full iteration after the AR they wait on,
                # so they never block the resid h1 loads behind them.
                nc.gpsimd.dma_start(out_e[lo:hi, :], ar2_out[lo:hi, :])

            # ---- schedule ----
            if NQ == 1:
                norm_chunk(0, xnf, x_e, False)
                qkw = load_tiles(qkw_e, QKF, DC)
                cosr = load_tiles(cos_e, T, 1)[0]
                sinr = load_tiles(sin_e, T, 1)[0]
                vw = load_tiles(vw_e, VF, DC)
                cmask = load_tiles(cm_e, QT, 1)[0]
                ow = load_tiles(ow_e, D, VF // 128)
                w2w = load_tiles(w2w_e, D, MIDC)
                qk_chunk(0)
                v_chunk(0)
                attn_chunk(0)
                oproj_chunk(0)
                ar1_fire(0, T)
                norm_chunk(0, hnf, ar1_out, True)
                mlp_chunk(0)
                w2_chunk(0)
                ar2_fire(0, T)
                final_piece(0, T)
            else:
                for c in range(NQ):
                    norm_chunk(c, xnf, x_e, False)
                    if c == 0:
                        qkw = load_tiles(qkw_e, QKF, DC)
                        cosr = load_tiles(cos_e, T, 1)[0]
                        sinr = load_tiles(sin_e, T, 1)[0]
                    qk_chunk(c)
                    if c == 0:
                        vw = load_tiles(vw_e, VF, DC)
                    v_chunk(c)
                    if c == 0:
                        cmask = load_tiles(cm_e, QT, 1)[0]
                    # previous chunk's resid norm + MLP ride inside the
                    # attention steps to keep the PE from draining while ACT
                    # computes the exps. Chunk 1 can't host chunk 0's units
                    # (AR1(0) isn't done yet mid-T(1)); later chunks cap the
                    # in-attention consumption so leftover units cover the
                    # AR1(c) ring after the fire.
                    if c >= 2:
                        k = c - 1
                        pre = [lambda k=k: norm_chunk(k, hnf, ar1_out, True)]
                        units = mlp_units(k)
                    else:
                        pre, units = [], []
                    attn_chunk(c, pre, units, max_units=8)
                    if c == 0:
                        ow = load_tiles(ow_e, D, VF // 128)
                    oproj_chunk(c)
                    ar1_fire(c * QT, (c + 1) * QT)
                    if c == 0:
                        w2w = load_tiles(w2w_e, D, MIDC)
                    if c == 1:
                        norm_chunk(0, hnf, ar1_out, True)
                        for u in mlp_units(0):
                            u()
                        ar2_fire(0, QT)
                    elif c >= 2:
                        k = c - 1
                        for u in units:  # spill units that didn't fit
                            u()
                        ar2_fire(k * QT, (k + 1) * QT)
                        final_piece((k - 1) * QT, k * QT)
                # tail: last chunk of MLP; W2/AR2 split in two pieces so the
                # first AR2 ring overlaps the second half's compute
                k = NQ - 1
                norm_chunk(k, hnf, ar1_out, True)
                for mc in range(MIDC):
                    mlp_mc(k, mc)
                h = k * CPQ + CPQ // 2
                for ti in range(k * CPQ, h):
                    w2_ti(ti)
                ar2_fire(k * QT, h * 128)
                for ti in range(h, (k + 1) * CPQ):
                    w2_ti(ti)
                ar2_fire(h * 128, (k + 1) * QT)
                final_piece((k - 1) * QT, k * QT)
                final_piece(k * QT, h * 128)
                final_piece(h * 128, (k + 1) * QT)

    nc.compile()
    return nc


def make_in_maps(x, n1_w, n2_w, qkv_w, o_w, w1_w, wg_w, w2_w, T):
    QT = min(512, T)
    CPQ = QT // 128
    half = DH // 2
    freqs = np.arange(half, dtype=np.float64) / half
    theta = 1.0 / ROPE_BASE**freqs
    ang = np.arange(T, dtype=np.float64)[:, None] * theta[None, :]  # [T, 32]
    p = np.arange(128) % half
    cosr = np.cos(ang)[:, p].T.astype(BF)  # [128, T]
    sinr = np.sin(ang)[:, p].T.astype(BF)
    cm = np.zeros((CPQ * 128, QT), dtype=BF)
    for j in range(CPQ):
        tk = np.arange(128)[:, None]
        tq = np.arange(QT)[None, :]
        cm[j * 128 : (j + 1) * 128] = (tq >= j * 128 + tk).astype(BF)

    in_maps = []
    for c in range(8):
        b, r = c // 4, c % 4
        qs = slice(r * VF, (r + 1) * VF)
        qr = qkv_w[0 * D :][qs] * n1_w[None, :]
        kr = qkv_w[1 * D :][qs] * n1_w[None, :]
        vr = qkv_w[2 * D :][qs] * n1_w[None, :]
        ms = slice(r * MID, (r + 1) * MID)
        in_maps.append(
            {
                "x": np.ascontiguousarray(x[b, :T], np.float32),
                "qkw_t": np.ascontiguousarray(
                    np.concatenate([qr, kr], 0).T.astype(BF)
                ),
                "vw_m": np.ascontiguousarray(vr.T.astype(BF)),
                "ow_m": np.ascontiguousarray(o_w[:, qs].T.astype(BF)),
                "w1w_t": np.ascontiguousarray(
                    (w1_w[ms] * n2_w[None, :]).T.astype(BF)
                ),
                "wgw_t": np.ascontiguousarray(
                    (wg_w[ms] * n2_w[None, :]).T.astype(BF)
                ),
                "w2w_m": np.ascontiguousarray(w2_w[:, ms].T.astype(BF)),
                "cosr": cosr,
                "sinr": sinr,
                "cmask": cm,
                "ident": np.eye(128, dtype=BF),
            }
        )
    return in_maps


_CACHE = {}


def _get_nc(T):
    if T not in _CACHE:
        _CACHE[T] = build_nc(T, use_silu=True)
    return _CACHE[T]


def run(inputs, T=2048, trace=False):
    nc = _get_nc(T)
    in_maps = make_in_maps(T=T, **inputs)
    res = run_bass_kernel_spmd(nc, in_maps, core_ids=list(range(8)), trace=trace)
    out = np.stack([res.results[0]["out"], res.results[4]["out"]])
    return out, res


def kernel(**inputs):
    out, _ = run(inputs, T=2048)
    return out


# revision 39
# speedup vs baseline: 1.0927x; 1.0273x over previous
"""Distributed Trainium2 kernel for a dense transformer block.

Reference computation (per batch):
  x = x + o_proj(attn(rope(qkv(rmsnorm(x))), causal)) ; x = x + w2(silu(wg(rmsnorm(x))) * w1(rmsnorm(x)))

Sharding: DP=2 on batch x TP=4 on heads / MLP rows (Megatron).
Cores 0-3 handle batch 0, cores 4-7 batch 1. Within a group, rank r owns
heads 4r..4r+3 and MLP rows 1024r..1024(r+1). Two bf16 AllReduces per
group, chunked and software-pipelined against compute.

v2 schedule: per-512-token-chunk pipeline
  A(c) norm+transpose -> B(c) qkv+rope -> V(c) -> T(c) attention (skewed
  score/exp/pv) -> O(c) oproj -> AR1(c); resid(c-1)+MLP(c-1)+AR2(c-1)
  interleaved between attention chunks. rsqrt via exp(-0.5*ln(m)) keeps
  the ACT engine on one table through the attention phase; attention
  row-sum reciprocals batched across partitions.
"""

import sys

sys.path.insert(0, "/opt/trn_rl_repo")

import numpy as np
import ml_dtypes

import concourse.bass as bass
import concourse.bacc as bacc
import concourse.mybir as mybir
import concourse.tile as tile
from concourse.bass_utils import run_bass_kernel_spmd

BF = ml_dtypes.bfloat16
F32 = mybir.dt.float32
BF16 = mybir.dt.bfloat16

D = 1024
NH = 16
DH = 64
MULT = 4
EPS = 1e-5
ROPE_BASE = 10000.0
B = 2
TP = 4  # tensor-parallel ranks per group
HPC = NH // TP  # heads per core = 4
QKF = 2 * HPC * DH  # q+k shard features = 512
VF = HPC * DH  # v shard features = 256
MID = MULT * D // TP  # mlp rows per core = 1024
AF = mybir.ActivationFunctionType
ALU = mybir.AluOpType


def build_nc(T, use_silu=True):
    """Build the SPMD graph for one core (token count T per batch)."""
    DC = D // 128  # d chunks = 8
    TT = T // 128  # token tiles
    QT = min(512, T)  # q-tile width == chunk width
    NQ = T // QT  # number of chunks
    CPQ = QT // 128  # 128-token tiles per chunk
    MIDC = MID // 128  # mlp row chunks = 8
    NT = D // 512

    nc = bacc.Bacc("TRN2", target_bir_lowering=False, debug=False, num_devices=8)

    x_e = nc.dram_tensor("x", [T, D], F32, kind="ExternalInput")
    qkw_e = nc.dram_tensor("qkw_t", [D, QKF], BF16, kind="ExternalInput")
    vw_e = nc.dram_tensor("vw_m", [D, VF], BF16, kind="ExternalInput")
    ow_e = nc.dram_tensor("ow_m", [VF, D], BF16, kind="ExternalInput")
    w1w_e = nc.dram_tensor("w1w_t", [D, MID], BF16, kind="ExternalInput")
    wgw_e = nc.dram_tensor("wgw_t", [D, MID], BF16, kind="ExternalInput")
    w2w_e = nc.dram_tensor("w2w_m", [MID, D], BF16, kind="ExternalInput")
    cos_e = nc.dram_tensor("cosr", [128, T], BF16, kind="ExternalInput")
    sin_e = nc.dram_tensor("sinr", [128, T], BF16, kind="ExternalInput")
    cm_e = nc.dram_tensor("cmask", [CPQ * 128, QT], BF16, kind="ExternalInput")
    id_e = nc.dram_tensor("ident", [128, 128], BF16, kind="ExternalInput")
    out_e = nc.dram_tensor("out", [T, D], F32, kind="ExternalOutput")

    groups = [[0, 1, 2, 3], [4, 5, 6, 7]]

    with tile.TileContext(nc) as tc:
        with (
            tc.tile_pool(name="const", bufs=1) as cpool,
            tc.tile_pool(name="actfm", bufs=1) as fmpool,
            tc.tile_pool(name="qko", bufs=1) as qkpool,
            tc.tile_pool(name="afm", bufs=1) as apool,
            tc.tile_pool(name="vaug", bufs=1) as vpool,
            tc.tile_pool(name="xin", bufs=2) as xpool,
            tc.tile_pool(name="xnb", bufs=2) as xnpool,
            tc.tile_pool(name="work", bufs=4) as wpool,
            tc.tile_pool(name="rope", bufs=1) as rpool,
            tc.tile_pool(name="stats", bufs=8) as spool,
            tc.tile_pool(name="sums", bufs=1) as supool,
            tc.tile_pool(name="psA", bufs=3, space="PSUM") as psA,
            tc.tile_pool(name="psO", bufs=2, space="PSUM") as psO,
            tc.tile_pool(name="psS", bufs=2, space="PSUM") as psS,
            tc.tile_pool(name="dram", bufs=1, space="DRAM") as dpool,
        ):
            # ---- resident weights / tables ----
            def load_tiles(src, width, n, dt=BF16):
                ts = []
                for i in range(n):
                    t = cpool.tile(
                        [128, width], dt, tag=f"{src.name}_{i}", name=f"{src.name}_{i}"
                    )
                    nc.sync.dma_start(t[:], src[i * 128 : (i + 1) * 128, :])
                    ts.append(t)
                return ts

            # only what chunk 0's norm needs up front; the big weight loads
            # are emitted just-in-time inside the schedule so the x DMAs and
            # first norm/transposes aren't queued behind them
            w1r = w1w_e.rearrange("(c p) m -> p c m", p=128)
            wgr = wgw_e.rearrange("(c p) m -> p c m", p=128)
            ones64 = cpool.tile([128, 64], BF16, tag="ones64", name="ones64")
            nc.vector.memset(ones64[:], 1.0)
            ident = load_tiles(id_e, 128, 1)[0]
            epsc = cpool.tile([128, 1], F32, tag="epsc", name="epsc")
            nc.vector.memset(epsc[:], EPS)

            ar1_in = dpool.tile([T, D], BF16, name="ar1_in")
            ar1_out = dpool.tile([T, D], BF16, name="ar1_out")
            ar2_in = dpool.tile([T, D], BF16, name="ar2_in")
            ar2_out = dpool.tile([T, D], BF16, name="ar2_out")

            # ---- persistent activation tiles ----
            # xnf chunk-c columns are consumed by qk/v of chunk c before the
            # resid norm overwrites them as hnf — one physical set serves both.
            xnf = [
                fmpool.tile([128, T], BF16, tag=f"fm{d}", name=f"xnf{d}")
                for d in range(DC)
            ]
            hnf = xnf
            q_sb = [
                qkpool.tile([128, T], BF16, tag=f"qk{i}", name=f"q{i}")
                for i in range(2)
            ]
            k_sb = [
                qkpool.tile([128, T], BF16, tag=f"qk{i + 2}", name=f"k{i}")
                for i in range(2)
            ]
            O_sb = [
                qkpool.tile([128, T], BF16, tag=f"qk{i + 4}", name=f"O{i}")
                for i in range(2)
            ]
            On_sb = [
                qkpool.tile([128, T], BF16, tag=f"qk{i + 6}", name=f"On{i}")
                for i in range(2)
            ]
            a_fm = [
                apool.tile([128, T], BF16, tag=f"am{d}", name=f"a{d}")
                for d in range(MIDC)
            ]
            v_aug = [
                vpool.tile([128, HPC, DH + 1], BF16, tag=f"va{ti}", name=f"va{ti}")
                for ti in range(TT)
            ]

            # ---- norm helper: one 512-token chunk into feature-major ----
            # rsqrt via batched DVE Newton iteration: ACT only runs Square
            # (present in every table) so the ACT table never reloads.
            def norm_chunk(c, fm_tiles, src_dram, use_gpsimd):
                xns = []
                xts = []
                msb = spool.tile([128, CPQ], F32, tag="msb", name="msb", bufs=2)
                for j in range(CPQ):
                    ti = c * CPQ + j
                    xt = xpool.tile([128, D], F32, tag="xt", name="xt", bufs=4)
                    if use_gpsimd:
                        nc.gpsimd.dma_start(
                            xt[:], src_dram[ti * 128 : (ti + 1) * 128, :]
                        )
                    else:
                        nc.sync.dma_start(
                            xt[:], src_dram[ti * 128 : (ti + 1) * 128, :]
                        )
                    xn = xnpool.tile([128, D], BF16, tag="xn", name="xn", bufs=4)
                    # square's elementwise output is scratch — reuse xn, which
                    # the normalize multiply overwrites right after
                    nc.scalar.activation(
                        out=xn[:],
                        in_=xt[:],
                        func=AF.Square,
                        accum_out=msb[:, j : j + 1],
                    )
                    xns.append(xn)
                    xts.append(xt)
                # m = ss/D + eps; y = rsqrt(m) by Newton from y0 = 1/m
                # (m in ~[0.5, 4] here so convergence is fast and safe)
                m4 = spool.tile([128, CPQ], F32, tag="m4", name="m4", bufs=2)
                nc.vector.tensor_scalar(
                    m4[:], msb[:], 1.0 / D, EPS, ALU.mult, ALU.add
                )
                y4 = spool.tile([128, CPQ], F32, tag="y4", name="y4", bufs=2)
                nc.vector.reciprocal(y4[:], m4[:])
                u4 = spool.tile([128, CPQ], F32, tag="u4", name="u4", bufs=2)
                for _ in range(4):
                    nc.vector.tensor_mul(u4[:], y4[:], y4[:])
                    nc.vector.tensor_mul(u4[:], u4[:], m4[:])
                    nc.vector.tensor_scalar(
                        u4[:], u4[:], -0.5, 1.5, ALU.mult, ALU.add
                    )
                    nc.vector.tensor_mul(y4[:], y4[:], u4[:])
                for j in range(CPQ):
                    nc.vector.tensor_scalar_mul(
                        xns[j][:], xts[j][:], y4[:, j : j + 1]
                    )
                for di in range(DC):
                    tp = psS.tile([128, QT], BF16, tag="tp", name="tp", bufs=2)
                    for j, xn in enumerate(xns):
                        nc.tensor.transpose(
                            tp[:, j * 128 : (j + 1) * 128],
                            xn[:, di * 128 : (di + 1) * 128],
                            ident[:],
                        )
                    nc.vector.tensor_copy(
                        fm_tiles[di][:, c * QT : (c + 1) * QT], tp[:]
                    )

            # ---- stage B: qkv + rope for one chunk ----
            def qk_chunk(c):
                tsl = slice(c * QT, (c + 1) * QT)
                for m in range(4):  # q01 q23 k01 k23
                    dst = q_sb[m] if m < 2 else k_sb[m - 2]
                    ps = psA.tile([128, QT], F32, tag="ps", name="ps")
                    for dc in range(DC):
                        nc.tensor.matmul(
                            ps[:, :QT],
                            qkw[dc][:, m * 128 : (m + 1) * 128],
                            xnf[dc][:, tsl],
                            start=(dc == 0),
                            stop=(dc == DC - 1),
                        )
                    qb = rpool.tile([128, QT], BF16, tag="qb", name="qb")
                    nc.scalar.copy(qb[:], ps[:, :QT])
                    rot = rpool.tile([128, QT], BF16, tag="rot", name="rot")
                    for hb in (0, 64):
                        nc.vector.tensor_scalar_mul(
                            rot[hb : hb + 32, :], qb[hb + 32 : hb + 64, :], -1.0
                        )
                        nc.vector.tensor_copy(
                            rot[hb + 32 : hb + 64, :], qb[hb : hb + 32, :]
                        )
                    nc.vector.tensor_mul(qb[:], qb[:], cosr[:, tsl])
                    nc.vector.tensor_mul(rot[:], rot[:], sinr[:, tsl])
                    nc.vector.tensor_add(dst[:, tsl], qb[:], rot[:])

            def v_chunk(c):
                for ti in range(c * CPQ, (c + 1) * CPQ):
                    ps = psS.tile([128, VF], F32, tag="tp", name="psv")
                    for dc in range(DC):
                        nc.tensor.matmul(
                            ps[:],
                            xnf[dc][:, ti * 128 : (ti + 1) * 128],
                            vw[dc][:],
                            start=(dc == 0),
                            stop=(dc == DC - 1),
                        )
                    va = v_aug[ti]
                    nc.vector.tensor_copy(
                        va[:, :, 0:DH], ps.rearrange("p (h d) -> p h d", h=HPC)
                    )
                    nc.vector.memset(va[:, :, DH : DH + 1], 1.0)

            # ---- attention for one chunk: skewed score/exp/pv pipeline ----
            # `pre_units`/`units` are closures of independent PE work (resid
            # norm, MLP matmul units of the previous chunk) interleaved into
            # the attention steps: they keep the PE fed while ACT computes
            # the exps, holding the PE clock at full p-state.
            SKEW = 2

            def attn_chunk(qt, pre_units=(), units=None, max_units=99):
                if units is None:
                    units = []
                popped = 0
                step = 0
                tsl = slice(qt * QT, (qt + 1) * QT)
                ncks = CPQ * (qt + 1)
                # row-sums of all 4 heads stacked at partitions 0/32/64/96
                sums = supool.tile([128, QT], F32, tag="sums", name="sums")
                for u in pre_units:
                    u()
                for hp in range(2):
                    opsP = [
                        psO.tile([DH + 1, QT], F32, tag="pso", name=f"ops{i}")
                        for i in range(2)
                    ]

                    def emit_score(ck):
                        pts = []
                        for i in range(2):
                            hb = i * 64
                            sp = psA.tile([128, QT], F32, tag="ps", name="sp")
                            nc.tensor.matmul(
                                sp[:, :QT],
                                k_sb[hp][hb : hb + DH, ck * 128 : (ck + 1) * 128],
                                q_sb[hp][hb : hb + DH, tsl],
                                start=True,
                                stop=True,
                            )
                            pt = wpool.tile(
                                [128, QT], BF16, tag="pt", name="pt", bufs=4
                            )
                            j = ck - CPQ * qt
                            if j > 0:
                                lo = j * 128
                                nc.vector.memset(pt[:, :lo], 0.0)
                                nc.scalar.activation(
                                    out=pt[:, lo:],
                                    in_=sp[:, lo:QT],
                                    func=AF.Exp,
                                    scale=0.125,
                                )
                                nc.vector.tensor_mul(
                                    pt[:, lo:], pt[:, lo:], cmask[:, : QT - lo]
                                )
                            else:
                                nc.scalar.activation(
                                    out=pt[:], in_=sp[:, :QT], func=AF.Exp, scale=0.125
                                )
                                if j == 0:
                                    nc.vector.tensor_mul(pt[:], pt[:], cmask[:])
                            pts.append(pt)
                        return pts

                    def emit_pv(ck, pts):
                        for i in range(2):
                            nc.tensor.matmul(
                                opsP[i][:],
                                v_aug[ck][:, 2 * hp + i, :],
                                pts[i][:],
                                start=(ck == 0),
                                stop=(ck == ncks - 1),
                            )

                    buf = {}
                    for ck in range(ncks):
                        buf[ck] = emit_score(ck)
                        if ck >= SKEW:
                            emit_pv(ck - SKEW, buf.pop(ck - SKEW))
                        # spread units across both hp loops (one per ~4
                        # steps) so the PE never drains while ACT runs exps
                        step += 1
                        if units and popped < max_units and step % 4 == 2:
                            units.pop(0)()
                            popped += 1
                    for ck in range(max(0, ncks - SKEW), ncks):
                        emit_pv(ck, buf.pop(ck))

                    for i in range(2):
                        h = 2 * hp + i
                        nc.scalar.copy(
                            sums[32 * h : 32 * h + 1, :], opsP[i][DH : DH + 1, :]
                        )
                        nc.scalar.copy(
                            O_sb[hp][i * 64 : i * 64 + DH, tsl], opsP[i][0:DH, :]
                        )
                # cover the recip->bb latency with a couple of units
                for _ in range(2):
                    if units:
                        units.pop(0)()
                # one batched in-place reciprocal for all 4 heads
                rin = sums
                nc.vector.reciprocal(rin[:], sums[:])
                # cast to bf16 with a partition shift so every head's row sits
                # at a legal matmul base partition (0 or 32)
                rinb = [
                    wpool.tile([128, QT], BF16, tag="pt", name=f"rinb{hp}", bufs=4)
                    for hp in range(2)
                ]
                nc.vector.tensor_copy(rinb[0][0:64, :], rin[0:64, :])
                nc.vector.tensor_copy(rinb[1][0:64, :], rin[64:128, :])
                for hp in range(2):
                    bb = psS.tile([128, QT], F32, tag="bb", name="bb", bufs=1)
                    for i in range(2):
                        nc.tensor.matmul(
                            bb[i * 64 : (i + 1) * 64, :QT],
                            ones64[32 * i : 32 * i + 1, :],
                            rinb[hp][32 * i : 32 * i + 1, :],
                            start=True,
                            stop=True,
                        )
                    nc.vector.tensor_mul(
                        On_sb[hp][:, tsl], O_sb[hp][:, tsl], bb[:, :QT]
                    )

            # ---- o-proj + AR1 input for one chunk ----
            def oproj_chunk(c):
                for ti in range(c * CPQ, (c + 1) * CPQ):
                    ob = wpool.tile([128, D], BF16, tag="ob", name="ob", bufs=2)
                    xo = xpool.tile([128, D], F32, tag="xo", name="xo")
                    nc.sync.dma_start(xo[:], x_e[ti * 128 : (ti + 1) * 128, :])
                    for nt in range(NT):
                        ps = psA.tile([128, QT], F32, tag="ps", name="ps")
                        for cc in range(VF // 128):
                            nc.tensor.matmul(
                                ps[:, :512],
                                On_sb[cc][:, ti * 128 : (ti + 1) * 128],
                                ow[cc][:, nt * 512 : (nt + 1) * 512],
                                start=(cc == 0),
                                stop=(cc == VF // 128 - 1),
                            )
                        nc.vector.scalar_tensor_tensor(
                            ob[:, nt * 512 : (nt + 1) * 512],
                            xo[:, nt * 512 : (nt + 1) * 512],
                            1.0 / TP,
                            ps[:, :512],
                            ALU.mult,
                            ALU.add,
                        )
                    nc.sync.dma_start(ar1_in[ti * 128 : (ti + 1) * 128, :], ob[:])

            def ar1_fire(lo, hi):
                nc.gpsimd.collective_compute(
                    "AllReduce",
                    ALU.add,
                    ins=[ar1_in[lo:hi, :].opt()],
                    outs=[ar1_out[lo:hi, :].opt()],
                    replica_groups=groups,
                )

            # ---- MLP for one chunk, as independent interleavable units ----
            def mlp_mc(c, mc):
                tsl = slice(c * QT, (c + 1) * QT)
                msl = slice(mc * 128, (mc + 1) * 128)
                wg_mc = wpool.tile(
                    [128, DC, 128], BF16, tag="wgs", name="wg_mc", bufs=2
                )
                nc.sync.dma_start(wg_mc[:], wgr[:, :, msl])
                w1_mc = wpool.tile(
                    [128, DC, 128], BF16, tag="w1s", name="w1_mc", bufs=2
                )
                nc.sync.dma_start(w1_mc[:], w1r[:, :, msl])
                psg = psA.tile([128, QT], F32, tag="ps", name="psg")
                for dc in range(DC):
                    nc.tensor.matmul(
                        psg[:, :QT],
                        wg_mc[:, dc, :],
                        hnf[dc][:, tsl],
                        start=(dc == 0),
                        stop=(dc == DC - 1),
                    )
                # silu via tanh (same ACT table as attention's exp, so the
                # interleave never reloads tables):
                #   silu(x) = x*0.5*(1+tanh(x/2)) -> g' = (tanh+1)*x; a = 0.5*g'*u
                th = rpool.tile([128, QT], BF16, tag="rot", name="th", bufs=1)
                nc.scalar.activation(
                    out=th[:], in_=psg[:, :QT], func=AF.Tanh, scale=0.5
                )
                g_sb = rpool.tile([128, QT], BF16, tag="qb", name="g2", bufs=1)
                nc.vector.scalar_tensor_tensor(
                    g_sb[:], th[:], 1.0, psg[:, :QT], ALU.add, ALU.mult
                )
                psu = psA.tile([128, QT], F32, tag="ps", name="psu")
                for dc in range(DC):
                    nc.tensor.matmul(
                        psu[:, :QT],
                        w1_mc[:, dc, :],
                        hnf[dc][:, tsl],
                        start=(dc == 0),
                        stop=(dc == DC - 1),
                    )
                nc.vector.scalar_tensor_tensor(
                    a_fm[mc][:, tsl], g_sb[:], 0.5, psu[:, :QT], ALU.mult, ALU.mult
                )

            def w2_ti(ti):
                ob = wpool.tile([128, D], BF16, tag="ob", name="ob", bufs=2)
                h1t = xpool.tile([128, D], F32, tag="h1t", name="h1t")
                nc.gpsimd.dma_start(h1t[:], ar1_out[ti * 128 : (ti + 1) * 128, :])
                for nt in range(NT):
                    ps = psA.tile([128, QT], F32, tag="ps", name="ps")
                    for mc in range(MIDC):
                        nc.tensor.matmul(
                            ps[:, :512],
                            a_fm[mc][:, ti * 128 : (ti + 1) * 128],
                            w2w[mc][:, nt * 512 : (nt + 1) * 512],
                            start=(mc == 0),
                            stop=(mc == MIDC - 1),
                        )
                    nc.vector.scalar_tensor_tensor(
                        ob[:, nt * 512 : (nt + 1) * 512],
                        h1t[:, nt * 512 : (nt + 1) * 512],
                        1.0 / TP,
                        ps[:, :512],
                        ALU.mult,
                        ALU.add,
                    )
                nc.sync.dma_start(ar2_in[ti * 128 : (ti + 1) * 128, :], ob[:])

            def mlp_units(c):
                us = [
                    (lambda mc=mc: mlp_mc(c, mc)) for mc in range(MIDC)
                ]
                us += [
                    (lambda ti=ti: w2_ti(ti))
                    for ti in range(c * CPQ, (c + 1) * CPQ)
                ]
                return us

            def mlp_chunk(c):
                for u in mlp_units(c):
                    u()

            def w2_chunk(c):
                pass

            def ar2_fire(lo, hi):
                nc.gpsimd.collective_compute(
                    "AllReduce",
                    ALU.add,
                    ins=[ar2_in[lo:hi, :].opt()],
                    outs=[ar2_out[lo:hi, :].opt()],
                    replica_groups=groups,
                )

            def final_piece(lo, hi):
                # gpsimd: the bf16->f32 cast DMA is gpsimd-only. Emission
                # points are one full iteration after the AR they wait on,
                # so they never block the resid h1 loads behind them.
                nc.gpsimd.dma_start(out_e[lo:hi, :], ar2_out[lo:hi, :])

            # ---- schedule ----
            if NQ == 1:
                norm_chunk(0, xnf, x_e, False)
                qkw = load_tiles(qkw_e, QKF, DC)
                cosr = load_tiles(cos_e, T, 1)[0]
                sinr = load_tiles(sin_e, T, 1)[0]
                vw = load_tiles(vw_e, VF, DC)
                cmask = load_tiles(cm_e, QT, 1)[0]
                ow = load_tiles(ow_e, D, VF // 128)
                w2w = load_tiles(w2w_e, D, MIDC)
                qk_chunk(0)
                v_chunk(0)
                attn_chunk(0)
                oproj_chunk(0)
                ar1_fire(0, T)
                norm_chunk(0, hnf, ar1_out, True)
                mlp_chunk(0)
                w2_chunk(0)
                ar2_fire(0, T)
                final_piece(0, T)
            else:

                def prep(c):
                    norm_chunk(c, xnf, x_e, False)
                    qk_chunk(c)
                    v_chunk(c)

                # chunk 0 with just-in-time weight loads
                norm_chunk(0, xnf, x_e, False)
                qkw = load_tiles(qkw_e, QKF, DC)
                cosr = load_tiles(cos_e, T, 1)[0]
                sinr = load_tiles(sin_e, T, 1)[0]
                qk_chunk(0)
                vw = load_tiles(vw_e, VF, DC)
                v_chunk(0)
                cmask = load_tiles(cm_e, QT, 1)[0]
                attn_chunk(0)
                ow = load_tiles(ow_e, D, VF // 128)
                oproj_chunk(0)
                ar1_fire(0, QT)
                w2w = load_tiles(w2w_e, D, MIDC)
                # chunk 1 (AR1(0) not done mid-T(1), so no interleave yet)
                prep(1)
                attn_chunk(1)
                oproj_chunk(1)
                ar1_fire(QT, 2 * QT)
                # steady state: prep(c) first (fills the AR window), then the
                # c-2 post block, then attention hosting chunk c-1's resid
                # norm + MLP units
                for c in range(2, NQ):
                    prep(c)
                    k = c - 2
                    if k == 0:
                        norm_chunk(0, hnf, ar1_out, True)
                        for u in mlp_units(0):
                            u()
                        ar2_fire(0, QT)
                    else:
                        for u in post_units:
                            u()
                        ar2_fire(k * QT, (k + 1) * QT)
                        final_piece((k - 1) * QT, k * QT)
                    kk = c - 1
                    pre = [lambda kk=kk: norm_chunk(kk, hnf, ar1_out, True)]
                    post_units = mlp_units(kk)
                    attn_chunk(
                        c, pre, post_units, max_units=4 if c == NQ - 1 else 8
                    )
                    oproj_chunk(c)
                    ar1_fire(c * QT, (c + 1) * QT)
                # leftover units of chunk NQ-2 cover the AR1(NQ-1) ring
                k = NQ - 2
                for u in post_units:
                    u()
                ar2_fire(k * QT, (k + 1) * QT)
                final_piece((k - 1) * QT, k * QT)
                # tail: last chunk of MLP; W2/AR2 split in two pieces so the
                # first AR2 ring overlaps the second half's compute
                k = NQ - 1
                norm_chunk(k, hnf, ar1_out, True)
                for mc in range(MIDC):
                    mlp_mc(k, mc)
                h = k * CPQ + CPQ // 2
                for ti in range(k * CPQ, h):
                    w2_ti(ti)
                ar2_fire(k * QT, h * 128)
                for ti in range(h, (k + 1) * CPQ):
                    w2_ti(ti)
                ar2_fire(h * 128, (k + 1) * QT)
                final_piece((k - 1) * QT, k * QT)
                final_piece(k * QT, h * 128)
                final_piece(h * 128, (k + 1) * QT)

    nc.compile()
    return nc


def make_in_maps(x, n1_w, n2_w, qkv_w, o_w, w1_w, wg_w, w2_w, T):
    QT = min(512, T)
    CPQ = QT // 128
    half = DH // 2
    freqs = np.arange(half, dtype=np.float64) / half
    theta = 1.0 / ROPE_BASE**freqs
    ang = np.arange(T, dtype=np.float64)[:, None] * theta[None, :]  # [T, 32]
    p = np.arange(128) % half
    cosr = np.cos(ang)[:, p].T.astype(BF)  # [128, T]
    sinr = np.sin(ang)[:, p].T.astype(BF)
    cm = np.zeros((CPQ * 128, QT), dtype=BF)
    for j in range(CPQ):
        tk = np.arange(128)[:, None]
        tq = np.arange(QT)[None, :]
        cm[j * 128 : (j + 1) * 128] = (tq >= j * 128 + tk).astype(BF)

    in_maps = []
    for c in range(8):
        b, r = c // 4, c % 4
        qs = slice(r * VF, (r + 1) * VF)
        qr = qkv_w[0 * D :][qs] * n1_w[None, :]
        kr = qkv_w[1 * D :][qs] * n1_w[None, :]
        vr = qkv_w[2 * D :][qs] * n1_w[None, :]
        ms = slice(r * MID, (r + 1) * MID)
        in_maps.append(
            {
                "x": np.ascontiguousarray(x[b, :T], np.float32),
                "qkw_t": np.ascontiguousarray(
                    np.concatenate([qr, kr], 0).T.astype(BF)
                ),
                "vw_m": np.ascontiguousarray(vr.T.astype(BF)),
                "ow_m": np.ascontiguousarray(o_w[:, qs].T.astype(BF)),
                "w1w_t": np.ascontiguousarray(
                    (w1_w[ms] * n2_w[None, :]).T.astype(BF)
                ),
                "wgw_t": np.ascontiguousarray(
                    (wg_w[ms] * n2_w[None, :]).T.astype(BF)
                ),
                "w2w_m": np.ascontiguousarray(w2_w[:, ms].T.astype(BF)),
                "cosr": cosr,
                "sinr": sinr,
                "cmask": cm,
                "ident": np.eye(128, dtype=BF),
            }
        )
    return in_maps


_CACHE = {}


def _get_nc(T):
    if T not in _CACHE:
        _CACHE[T] = build_nc(T, use_silu=True)
    return _CACHE[T]


def run(inputs, T=2048, trace=False):
    nc = _get_nc(T)
    in_maps = make_in_maps(T=T, **inputs)
    res = run_bass_kernel_spmd(nc, in_maps, core_ids=list(range(8)), trace=trace)
    out = np.stack([res.results[0]["out"], res.results[4]["out"]])
    return out, res


def kernel(**inputs):
    out, _ = run(inputs, T=2048)
    return out
